# revision 3
# baseline (speedup 1.0000x reference)
"""Trainium2 Bass kernel for MinimalKAN forward (nn_MinimalKAN_Normalized).

Math:
  a = sigmoid(alpha)
  out = (1-a) * (x @ W.T + b) + (a/sqrt(I)) * (x @ C0 + x^2 @ C1 + x^3 @ C2)

Folding the alpha blend into the weights on the host gives exactly
  out = x @ A + x^2 @ B + x^3 @ C + b_eff
with A = (1-a) W.T + s C0, B = s C1, C = s C2, b_eff = (1-a) b, s = a/sqrt(I).

Device strategy (data-parallel over batch, 8 cores), per core 4096 rows.
The contraction index i sits on SBUF partitions; the host feeds x^T in fp16.
Mixed precision split by term magnitude:
  - linear term x @ A: fp16 matmuls, 4 accumulating matmuls per 128-row tile.
  - kan terms x^2 @ B + x^3 @ C: fp8(e4m3) matmuls in DoubleRow perf mode:
    lhsT [128, 2, 128] loads two k-planes, K=256 per instruction.
Both weight sets are host-scaled by the SAME S=4096 (fp16 linear weights stay
well inside fp16 range: |A|*4096 < ~90; fp8 kan weights as before), so the
linear and kan matmuls accumulate into ONE PSUM bank per tile and a single
DVE scalar_tensor_tensor does merge+descale+bias: out = psum/4096 + b (fp16).

Per tile the 4 fp16 and 4 DR matmuls are INTERLEAVED (f16,DR,f16,DR,...):
a DoubleRow LDWEIGHTS (256 cols, no FWL) does not hide under a preceding
DR matmul (trace: alternating DR durs 379/566, +187ns ~= the DR LDW), but
it does hide under an fp16 matmul's 512-cycle stream.

Head: no dummy warmup matmuls -- group 0's real linear matmuls run during
the HAM cold window instead.  DMA kick order puts the critical set first:
scalar ring kicks wlin in two 256KB halves then wkan (weights land ~9.5us),
sync ring kicks x tiles of group 0 per-tile then bias.  Group 0 runs its
16 linear matmuls k-outer (matmul (j,k) needs x tile j and wl slice k, both
arriving in that order) while wkan/basis land, then its per-tile kan chains.
Tail: the last group drains its outputs per-tile to shorten the post-matmul
critical path (single STT + small DMA before the fixed NEFF epilogue).
"""

import os
import numpy as np

import concourse.bass as bass
from concourse import bacc
import concourse.mybir as mybir
import concourse.tile as tile
from concourse.bass_utils import run_bass_kernel_spmd

N_CORES = 8
B, I, O = 32768, 512, 512
BS = B // N_CORES          # rows per core
P = 128
KS = I // P                # 4 contraction k-tiles per basis
N_TILES = BS // P          # 32 x 128-row tiles per core
G = 4                      # tiles per x^2/x^3 group
N_GROUPS = N_TILES // G

S = 4096.0                 # shared host weight scale (fp16 linear + fp8 kan)


def _build(repeat: int = 1) -> bass.Bass:
    f16 = mybir.dt.float16
    f8 = mybir.dt.float8e4
    f32 = mybir.dt.float32
    sq = mybir.ActivationFunctionType.Square
    DR = mybir.MatmulPerfMode.DoubleRow
    mult = mybir.AluOpType.mult
    add = mybir.AluOpType.add

    nc = bacc.Bacc("TRN2", target_bir_lowering=False, debug=False,
                   num_devices=N_CORES)

    x_d = nc.dram_tensor("xt", [P, N_TILES, KS, P], f16,
                         kind="ExternalInput")
    wl_d = nc.dram_tensor("wlin", [P, KS, O], f16, kind="ExternalInput")
    wk_d = nc.dram_tensor("wkan", [P, 2 * KS, O], f8, kind="ExternalInput")
    b_d = nc.dram_tensor("bias", [P, O], f16, kind="ExternalInput")
    o_d = nc.dram_tensor("out", [P, N_TILES, O], f16,
                         kind="ExternalOutput")

    with tile.TileContext(nc) as tc:
        with (
            tc.tile_pool(name="const", bufs=1) as const,
            tc.tile_pool(name="xin", bufs=3) as xin,
            tc.tile_pool(name="basis", bufs=3) as basis,
            tc.tile_pool(name="outp", bufs=3) as outp,
            tc.tile_pool(name="ps", bufs=6, space="PSUM") as ps,
        ):
            # weights on the scalar (ACT) ring, critical-first: wlin halves
            # (first linear matmuls need k0/k1 earliest), then wkan.
            wl_sb = const.tile([P, KS, O], f16)
            nc.scalar.dma_start(wl_sb[:, 0:2, :], wl_d[:, 0:2, :])
            nc.scalar.dma_start(wl_sb[:, 2:4, :], wl_d[:, 2:4, :])
            wk_sb = const.tile([P, 2 * KS, O], f8)
            nc.scalar.dma_start(wk_sb[:], wk_d[:])
            bsb = const.tile([P, O], f16)

            for g in [i for _ in range(repeat) for i in range(N_GROUPS)]:
                xT = xin.tile([P, G, KS, P], f16, tag="xT")
                if g == 0:
                    # per-tile kicks so compute starts after the first 128KB
                    for j in range(G):
                        nc.sync.dma_start(xT[:, j], x_d[:, g * G + j])
                    nc.sync.dma_start(bsb[:], b_d[:])
                else:
                    nc.sync.dma_start(xT[:], x_d[:, g * G:(g + 1) * G])
                b8 = basis.tile([P, G, 2 * KS, P], f8, tag="b8")
                o_sb = outp.tile([P, G, O], f16, tag="o_sb")
                if g == 0:
                    # per-tile basis ops so the first kan matmuls don't
                    # wait on the whole group's x^T DMA
                    for j in range(G):
                        nc.scalar.activation(b8[:, j, 0:KS, :],
                                             xT[:, j], sq)
                        nc.gpsimd.tensor_mul(b8[:, j, KS:2 * KS, :],
                                             b8[:, j, 0:KS, :], xT[:, j])
                    # k-outer linear sweep: matmul (j,k) needs only x tile
                    # j (landing in j order) and wl slice k; runs cold
                    # during the HAM warmup window doing real work.
                    banks = [ps.tile([P, O], f32, tag="po", name=f"po_g0_{j}")
                             for j in range(G)]
                    for k in range(KS):
                        for j in range(G):
                            nc.tensor.matmul(
                                banks[j][:], xT[:, j, k, :], wl_sb[:, k, :],
                                start=(k == 0), stop=False,
                                skip_group_check=True)
                    for j in range(G):
                        for t in range(KS):
                            nc.tensor.matmul(
                                banks[j][:],
                                b8[:, j, 2 * t:2 * t + 2, :],
                                wk_sb[:, 2 * t:2 * t + 2, :],
                                start=False, stop=(t == KS - 1),
                                perf_mode=DR, skip_group_check=True)
                        nc.vector.scalar_tensor_tensor(
                            o_sb[:, j, :], banks[j][:], 1.0 / S, bsb[:],
                            mult, add)
                    nc.scalar.dma_start(o_d[:, 0:G, :], o_sb[:])
                    continue
                nc.scalar.activation(b8[:, :, 0:KS, :], xT[:], sq)
                nc.gpsimd.tensor_mul(b8[:, :, KS:2 * KS, :],
                                     b8[:, :, 0:KS, :], xT[:])
                for j in range(G):
                    po = ps.tile([P, O], f32, tag="po")
                    # interleave f16 and DR so each DR LDWEIGHTS hides
                    # under the preceding fp16 matmul's stream
                    for k in range(KS):
                        nc.tensor.matmul(
                            po[:], xT[:, j, k, :], wl_sb[:, k, :],
                            start=(k == 0), stop=False,
                            skip_group_check=True)
                        nc.tensor.matmul(
                            po[:],
                            b8[:, j, 2 * k:2 * k + 2, :],
                            wk_sb[:, 2 * k:2 * k + 2, :],
                            start=False, stop=(k == KS - 1),
                            perf_mode=DR, skip_group_check=True)
                    nc.vector.scalar_tensor_tensor(
                        o_sb[:, j, :], po[:], 1.0 / S, bsb[:], mult, add)
                    if g == N_GROUPS - 1:
                        # last group drains per-tile to shorten the tail
                        nc.scalar.dma_start(o_d[:, g * G + j, :],
                                            o_sb[:, j, :])
                if g != N_GROUPS - 1:
                    nc.scalar.dma_start(o_d[:, g * G:(g + 1) * G, :],
                                        o_sb[:])

    nc.compile()
    return nc


_NC_CACHE: dict[int, bass.Bass] = {}


def _get_nc(repeat: int = 1) -> bass.Bass:
    nc = _NC_CACHE.get(repeat)
    if nc is None:
        nc = _build(repeat)
        _NC_CACHE[repeat] = nc
    return nc


def _fold_weights(coeffs, W, b, alpha):
    a = 1.0 / (1.0 + np.exp(-np.float64(alpha)))
    s = a / np.sqrt(np.float64(I))
    A = (1.0 - a) * W.astype(np.float64).T + s * coeffs[:, :, 0].astype(np.float64)
    Bm = s * coeffs[:, :, 1].astype(np.float64)
    Cm = s * coeffs[:, :, 2].astype(np.float64)
    # [I, O] -> [P, KS, O] with row ks*P+p on partition p, slot ks
    wlin = (A * S).astype(np.float16)
    wlin = np.ascontiguousarray(
        wlin.reshape(KS, P, O).transpose(1, 0, 2))
    f8np = mybir.dt.np(mybir.dt.float8e4)
    wkan = np.concatenate([Bm * S, Cm * S], axis=0)
    wkan = np.clip(wkan, -240.0, 240.0).astype(f8np)
    wkan = np.ascontiguousarray(
        wkan.reshape(2 * KS, P, O).transpose(1, 0, 2))
    b_eff = ((1.0 - a) * b.astype(np.float64)).astype(np.float16)
    bias_rep = np.ascontiguousarray(
        np.broadcast_to(b_eff[None, :], (P, O)))
    return wlin, wkan, bias_rep


def _make_in_maps(x, coeffs, W, b, alpha):
    wlin, wkan, bias_rep = _fold_weights(coeffs, W, b, alpha)
    x = np.asarray(x, dtype=np.float32)
    in_maps = []
    for c in range(N_CORES):
        shard = x[c * BS:(c + 1) * BS].astype(np.float16)
        # [BS, I] -> [P, N_TILES, KS, P]: xt[p, t, ks, c'] =
        # x[t*P+c', ks*P+p]
        xt = np.ascontiguousarray(
            shard.reshape(N_TILES, P, KS, P).transpose(3, 0, 2, 1))
        in_maps.append({
            "wlin": wlin, "wkan": wkan, "bias": bias_rep, "xt": xt,
        })
    return in_maps


def _unpack_out(raw):
    # [P, N_TILES, O] fp16 -> [BS, O] f32: row t*P + p
    return np.ascontiguousarray(
        np.asarray(raw).astype(np.float32).transpose(1, 0, 2)
    ).reshape(BS, O)


def _run(x, coeffs, W, b, alpha, trace=False):
    nc = _get_nc()
    in_maps = _make_in_maps(x, coeffs, W, b, alpha)
    res = run_bass_kernel_spmd(nc, in_maps, core_ids=list(range(N_CORES)),
                               trace=trace)
    out = np.concatenate([_unpack_out(r["out"]) for r in res.results], axis=0)
    return out, res


def kernel(x, coeffs, W, b, alpha):
    out, _ = _run(x, coeffs, W, b, alpha, trace=False)
    return out


# revision 4
# speedup vs baseline: 1.0495x; 1.0495x over previous
"""Trainium2 Bass kernel for MinimalKAN forward (nn_MinimalKAN_Normalized).

Math:
  a = sigmoid(alpha)
  out = (1-a) * (x @ W.T + b) + (a/sqrt(I)) * (x @ C0 + x^2 @ C1 + x^3 @ C2)

Folding the alpha blend into the weights on the host gives exactly
  out = x @ A + x^2 @ B + x^3 @ C + b_eff
with A = (1-a) W.T + s C0, B = s C1, C = s C2, b_eff = (1-a) b, s = a/sqrt(I).

Device strategy (data-parallel over batch, 8 cores), per core 4096 rows.
The contraction index i sits on SBUF partitions; the host feeds x^T in fp16.
Mixed precision split by term magnitude (fp8 for the linear term fails the
2e-2 gate: measured 3.1e-2 all-fp8, 2.3e-2 half-fp8):
  - linear term x @ A: fp16 matmuls, 4 accumulating matmuls per 128-row tile.
  - kan terms x^2 @ B + x^3 @ C: fp8(e4m3) matmuls in DoubleRow perf mode:
    lhsT [128, 2, 128] loads two k-planes, K=256 per instruction.
Both weight sets are host-scaled by the SAME S=4096 (fp16 linear weights stay
well inside fp16 range: |A|*4096 < ~90), so linear and kan matmuls accumulate
into ONE PSUM bank per tile and a single DVE scalar_tensor_tensor does
merge+descale+bias: out = psum/4096 + b (fp16 out).

PE schedule is a flat software pipeline over the 32 tiles with a 2-tile
stagger: tile t's four fp16 matmuls interleave one-for-one with tile
(t-2)'s four DR matmuls.  A DoubleRow LDWEIGHTS (256 cols, no FWL) does
not hide under a preceding DR matmul (trace: +187ns on every DR-after-DR)
but does hide under an fp16 matmul's 512-cycle stream; the stagger keeps
the f16/DR alternation across group boundaries.  Steady state measured
~437ns per (f16,DR) pair = issue-rate roofline for N=512.

Head: ~20 dummy 128-col matmuls bridge the initial DMA fill (PE busy from
~7.2us so the HAM clock-gate window expires before real matmuls start);
first kicks are 256KB halves (x tiles 0-1 / 2-3 on the sync ring; wlin
halves then wkan halves on the scalar ring) so the first real matmul can
start ~9.5us.  Tail: last group drains per-tile.
"""

import os
import numpy as np

import concourse.bass as bass
from concourse import bacc
import concourse.mybir as mybir
import concourse.tile as tile
from concourse.bass_utils import run_bass_kernel_spmd

N_CORES = 8
B, I, O = 32768, 512, 512
BS = B // N_CORES          # rows per core
P = 128
KS = I // P                # 4 contraction k-tiles per basis
N_TILES = BS // P          # 32 x 128-row tiles per core
G = 4                      # tiles per x^2/x^3 group
N_GROUPS = N_TILES // G
LAG = 2                    # DR matmuls trail fp16 matmuls by this many tiles

S = 4096.0                 # shared host weight scale (fp16 linear + fp8 kan)
N_WARM = int(os.environ.get("KAN_WARM", "20"))


def _build(repeat: int = 1) -> bass.Bass:
    f16 = mybir.dt.float16
    f8 = mybir.dt.float8e4
    f32 = mybir.dt.float32
    sq = mybir.ActivationFunctionType.Square
    DR = mybir.MatmulPerfMode.DoubleRow
    mult = mybir.AluOpType.mult
    add = mybir.AluOpType.add

    nc = bacc.Bacc("TRN2", target_bir_lowering=False, debug=False,
                   num_devices=N_CORES)

    x_d = nc.dram_tensor("xt", [P, N_TILES, KS, P], f16,
                         kind="ExternalInput")
    wl_d = nc.dram_tensor("wlin", [P, KS, O], f16, kind="ExternalInput")
    wk_d = nc.dram_tensor("wkan", [P, 2 * KS, O], f8, kind="ExternalInput")
    b_d = nc.dram_tensor("bias", [P, O], f16, kind="ExternalInput")
    o_d = nc.dram_tensor("out", [P, N_TILES, O], f16,
                         kind="ExternalOutput")

    with tile.TileContext(nc) as tc:
        with (
            tc.tile_pool(name="const", bufs=1) as const,
            tc.tile_pool(name="xin", bufs=3) as xin,
            tc.tile_pool(name="basis", bufs=3) as basis,
            tc.tile_pool(name="outp", bufs=3) as outp,
            tc.tile_pool(name="ps", bufs=6, space="PSUM") as ps,
            tc.tile_pool(name="ps_w", bufs=1, space="PSUM") as ps_w,
        ):
            # weights on the scalar (ACT) ring, critical-first halves so the
            # first consumers unblock as early as possible
            wl_sb = const.tile([P, KS, O], f16)
            nc.scalar.dma_start(wl_sb[:, 0:2, :], wl_d[:, 0:2, :])
            nc.scalar.dma_start(wl_sb[:, 2:4, :], wl_d[:, 2:4, :])
            wk_sb = const.tile([P, 2 * KS, O], f8)
            nc.scalar.dma_start(wk_sb[:, 0:KS, :], wk_d[:, 0:KS, :])
            nc.scalar.dma_start(wk_sb[:, KS:2 * KS, :], wk_d[:, KS:2 * KS, :])
            bsb = const.tile([P, O], f16)

            # PE p-state warmup bridging the initial DMA fill (the HAM
            # clock-gate needs ~3.4us of sustained PE activity; dummy
            # 128-col matmuls are ~107ns each, results discarded)
            warm = const.tile([P, P], f16)
            nc.vector.memset(warm[:], 0.0)
            po_w = ps_w.tile([P, P], f32, tag="po_w")
            for _ in range(N_WARM):
                nc.tensor.matmul(po_w[:], warm[:], warm[:],
                                 start=True, stop=True,
                                 skip_group_check=True)

            for rep in range(repeat):
                xts = {}
                b8s = {}
                osbs = {}
                banks = {}

                def group_input(g):
                    xT = xin.tile([P, G, KS, P], f16, name=f"xT_{g}",
                                  tag="xT")
                    xts[g] = xT
                    if g == 0:
                        # first-group halves so compute starts ~1us earlier
                        nc.sync.dma_start(xT[:, 0:2], x_d[:, 0:2])
                        nc.sync.dma_start(xT[:, 2:4], x_d[:, 2:4])
                        nc.sync.dma_start(bsb[:], b_d[:])
                    else:
                        nc.sync.dma_start(xT[:], x_d[:, g * G:(g + 1) * G])
                    b8 = basis.tile([P, G, 2 * KS, P], f8, name=f"b8_{g}",
                                    tag="b8")
                    b8s[g] = b8
                    if g == 0:
                        # per-tile basis ops so the first DR matmuls don't
                        # wait on the whole group's square/cube
                        for j in range(G):
                            nc.scalar.activation(b8[:, j, 0:KS, :],
                                                 xT[:, j], sq)
                            nc.gpsimd.tensor_mul(b8[:, j, KS:2 * KS, :],
                                                 b8[:, j, 0:KS, :],
                                                 xT[:, j])
                    else:
                        nc.scalar.activation(b8[:, :, 0:KS, :], xT[:], sq)
                        nc.gpsimd.tensor_mul(b8[:, :, KS:2 * KS, :],
                                             b8[:, :, 0:KS, :], xT[:])
                    osbs[g] = outp.tile([P, G, O], f16, name=f"o_{g}",
                                        tag="o_sb")

                for t in range(N_TILES + LAG):
                    lt = t           # tile doing its fp16 linear matmuls
                    dt_ = t - LAG    # tile doing its DR kan matmuls
                    if lt < N_TILES:
                        gl, jl = divmod(lt, G)
                        if jl == 0:
                            group_input(gl)
                        banks[lt] = ps.tile([P, O], f32, name=f"po_{lt}",
                                            tag="po")
                    for k in range(KS):
                        if lt < N_TILES:
                            gl, jl = divmod(lt, G)
                            nc.tensor.matmul(
                                banks[lt][:], xts[gl][:, jl, k, :],
                                wl_sb[:, k, :],
                                start=(k == 0), stop=False,
                                skip_group_check=True)
                        if dt_ >= 0:
                            gd, jd = divmod(dt_, G)
                            nc.tensor.matmul(
                                banks[dt_][:],
                                b8s[gd][:, jd, 2 * k:2 * k + 2, :],
                                wk_sb[:, 2 * k:2 * k + 2, :],
                                start=False, stop=(k == KS - 1),
                                perf_mode=DR, skip_group_check=True)
                    if dt_ >= 0:
                        gd, jd = divmod(dt_, G)
                        nc.vector.scalar_tensor_tensor(
                            osbs[gd][:, jd, :], banks.pop(dt_)[:], 1.0 / S,
                            bsb[:], mult, add)
                        if gd == N_GROUPS - 1:
                            # last group drains per-tile: short tail
                            nc.scalar.dma_start(
                                o_d[:, gd * G + jd, :], osbs[gd][:, jd, :])
                        elif jd == G - 1:
                            nc.scalar.dma_start(
                                o_d[:, gd * G:(gd + 1) * G, :], osbs[gd][:])

    nc.compile()
    return nc


_NC_CACHE: dict[int, bass.Bass] = {}


def _get_nc(repeat: int = 1) -> bass.Bass:
    nc = _NC_CACHE.get(repeat)
    if nc is None:
        nc = _build(repeat)
        _NC_CACHE[repeat] = nc
    return nc


def _fold_weights(coeffs, W, b, alpha):
    a = 1.0 / (1.0 + np.exp(-np.float64(alpha)))
    s = a / np.sqrt(np.float64(I))
    A = (1.0 - a) * W.astype(np.float64).T + s * coeffs[:, :, 0].astype(np.float64)
    Bm = s * coeffs[:, :, 1].astype(np.float64)
    Cm = s * coeffs[:, :, 2].astype(np.float64)
    # [I, O] -> [P, KS, O] with row ks*P+p on partition p, slot ks
    wlin = (A * S).astype(np.float16)
    wlin = np.ascontiguousarray(
        wlin.reshape(KS, P, O).transpose(1, 0, 2))
    f8np = mybir.dt.np(mybir.dt.float8e4)
    wkan = np.concatenate([Bm * S, Cm * S], axis=0)
    wkan = np.clip(wkan, -240.0, 240.0).astype(f8np)
    wkan = np.ascontiguousarray(
        wkan.reshape(2 * KS, P, O).transpose(1, 0, 2))
    b_eff = ((1.0 - a) * b.astype(np.float64)).astype(np.float16)
    bias_rep = np.ascontiguousarray(
        np.broadcast_to(b_eff[None, :], (P, O)))
    return wlin, wkan, bias_rep


def _make_in_maps(x, coeffs, W, b, alpha):
    wlin, wkan, bias_rep = _fold_weights(coeffs, W, b, alpha)
    x = np.asarray(x, dtype=np.float32)
    in_maps = []
    for c in range(N_CORES):
        shard = x[c * BS:(c + 1) * BS].astype(np.float16)
        # [BS, I] -> [P, N_TILES, KS, P]: xt[p, t, ks, c'] =
        # x[t*P+c', ks*P+p]
        xt = np.ascontiguousarray(
            shard.reshape(N_TILES, P, KS, P).transpose(3, 0, 2, 1))
        in_maps.append({
            "wlin": wlin, "wkan": wkan, "bias": bias_rep, "xt": xt,
        })
    return in_maps


def _unpack_out(raw):
    # [P, N_TILES, O] fp16 -> [BS, O] f32: row t*P + p
    return np.ascontiguousarray(
        np.asarray(raw).astype(np.float32).transpose(1, 0, 2)
    ).reshape(BS, O)


def _run(x, coeffs, W, b, alpha, trace=False):
    nc = _get_nc()
    in_maps = _make_in_maps(x, coeffs, W, b, alpha)
    res = run_bass_kernel_spmd(nc, in_maps, core_ids=list(range(N_CORES)),
                               trace=trace)
    out = np.concatenate([_unpack_out(r["out"]) for r in res.results], axis=0)
    return out, res


def kernel(x, coeffs, W, b, alpha):
    out, _ = _run(x, coeffs, W, b, alpha, trace=False)
    return out
